# revision 1
# baseline (speedup 1.0000x reference)
"""DeformableConvBlock on 8 Trainium2 NeuronCores.

Per-core (data-parallel over batch, 1 image per core):
  1. offset conv (3x3, pad 1) via 9 accumulated PE matmuls on a zero-padded image
  2. bilinear sample indices/weights derived on-device (DVE/ACT); floor via
     round-to-nearest-even int16 cast with an epsilon guard
  3. 4-neighbor gather via GPSIMD ap_gather (d=4 interleaved padded buffer)
  4. bilinear weights replicated across partitions via ones-matmul into PSUM,
     applied on DVE; the 4-neighbor reduction rides PSUM accumulation of the
     deform GEMM (4 matmuls per tap into the same PSUM region)
  5. BN stats per channel via bn_stats/bn_aggr, all-reduced across the 8 cores
     (b_def cancels inside training-mode BN and is skipped)
  6. shortcut 1x1 conv on PE; BN affine + shortcut + ReLU fused on DVE/ACT
"""
import numpy as np
import ml_dtypes
from contextlib import ExitStack

import concourse.bass as bass
import concourse.bacc as bacc
import concourse.tile as tile
import concourse.mybir as mybir
from concourse.bass_utils import run_bass_kernel_spmd
from bass_rust import ScopedClock

F32 = mybir.dt.float32
BF16 = mybir.dt.bfloat16
I16 = mybir.dt.int16
AF = mybir.ActivationFunctionType
ALU = mybir.AluOpType

B, CIN, COUT, H, W = 8, 128, 256, 64, 64
HWP = H * W            # 4096
PADG = 3               # padded grid ring for the gather (offsets clamped to +-3)
GP = H + 2 * PADG      # 70
NPIX = GP * GP         # 4900
EPS = 1e-5
FLOOR_EPS = 0.499969482421875  # cast rounds-to-nearest-even; v-EPS rounds to floor(v)
CLAMP_LO = 0.001
CLAMP_HI = 68.999


def _patched_drain_and_barrier(self, tick_clock, wait_clock):
    # This walrus build rejects >1 sync-wait on a CTRL Drain; spread the tail
    # drain's waits over preceding sequencer nops.
    nc = self.nc
    drain_inst = nc.sync.drain()
    wait_clock.add_sem_waits(drain_inst.ins, ScopedClock({None: tick_clock.global_clock}))
    si = drain_inst.ins.sync_info
    if si is not None and si.on_wait and len(si.on_wait) > 1:
        waits = list(si.on_wait)
        bb = nc.cur_bb.bb
        assert bb.instructions[-1].name == drain_inst.ins.name
        bb.instructions.pop()
        for w in waits[1:]:
            nop = nc.sync.nop()
            nsi = nop.ins.sync_info
            if nsi is None:
                nop.ins.sync_info = mybir.SyncInfo(on_wait=[w], on_update=[])
            else:
                nsi.on_wait = list(nsi.on_wait) + [w]
        si.on_wait = waits[:1]
        bb.add_instruction(drain_inst.ins)
    nc.all_engine_barrier()
    assert self.sems is not None
    popped = nc._tile_sem_poison_stack.pop()
    assert popped is self._sem_poison
    nc.clear_and_free_semaphores(list(self.sems.allocated().values()))
    nc.all_engine_barrier()


tile.TileContext._drain_and_barrier = _patched_drain_and_barrier


_BUILD_OPTS = {"skip_gather": False, "skip_repl_mul": False, "skip_gemm": False,
               "skip_collective": False, "skip_wst": False}


def _build_program():
    nc = bacc.Bacc("TRN2", target_bir_lowering=False, debug=False, num_devices=8)

    x_in = nc.dram_tensor("x", [128, HWP], F32, kind="ExternalInput").ap()
    wof_in = nc.dram_tensor("wof", [128, 9, 18], BF16, kind="ExternalInput").ap()
    boff_in = nc.dram_tensor("boff", [18, 1], F32, kind="ExternalInput").ap()
    wdef_in = nc.dram_tensor("wdef", [128, 9, 256], BF16, kind="ExternalInput").ap()
    wsc_in = nc.dram_tensor("wsc", [128, 256], BF16, kind="ExternalInput").ap()
    cb3_in = nc.dram_tensor("cb3", [36, 2048], F32, kind="ExternalInput").ap()
    ones_in = nc.dram_tensor("onesb", [1, 128], BF16, kind="ExternalInput").ap()
    gam_in = nc.dram_tensor("gamma2", [128, 2], F32, kind="ExternalInput").ap()
    bsc_in = nc.dram_tensor("betasc2", [128, 2], F32, kind="ExternalInput").ap()

    out_d = nc.dram_tensor("out", [2, 128, HWP], F32, kind="ExternalOutput").ap()

    with tile.TileContext(nc) as tc, ExitStack() as ctx:
        singles = ctx.enter_context(tc.tile_pool(name="singles", bufs=1))
        dram = ctx.enter_context(tc.tile_pool(name="dram", bufs=1, space="DRAM"))

        # ---- constant loads (persistent) ----
        wof_sb = singles.tile([128, 9, 18], BF16)
        nc.sync.dma_start(out=wof_sb, in_=wof_in)
        wdef_sb = singles.tile([128, 9, 256], BF16)
        nc.sync.dma_start(out=wdef_sb, in_=wdef_in)
        wsc_sb = singles.tile([128, 256], BF16)
        nc.sync.dma_start(out=wsc_sb, in_=wsc_in)
        ones_sb = singles.tile([1, 128], BF16)
        nc.sync.dma_start(out=ones_sb, in_=ones_in)
        gam_sb = singles.tile([128, 2], F32)
        nc.sync.dma_start(out=gam_sb, in_=gam_in)
        bsc_sb = singles.tile([128, 2], F32)
        nc.sync.dma_start(out=bsc_sb, in_=bsc_in)
        eps_sb = singles.tile([128, 1], F32)
        nc.vector.memset(eps_sb, EPS)

        # zero the padded buffer FIRST (no deps) so it overlaps the input DMA
        pa_cm = tc.tile_pool(name="pa", bufs=1)
        pa = pa_cm.__enter__()
        xpad = pa.tile([128, NPIX + 76], BF16)  # 4900 + tail for shifted reads
        nc.vector.memset(xpad, 0.0)

        # ---- input image: f32 -> bf16 (xf in its own scope) ----
        with tc.tile_pool(name="xfp", bufs=1) as xfp:
            xf = xfp.tile([128, HWP], F32)
            nc.sync.dma_start(out=xf, in_=x_in)
            xbf = singles.tile([128, HWP], BF16)
            nc.scalar.activation(out=xbf, in_=xf, func=AF.Copy)

        boff_sb = pa.tile([18, 1], F32)
        nc.sync.dma_start(out=boff_sb, in_=boff_in)
        cb3_sb = pa.tile([36, 2048], F32)
        nc.sync.dma_start(out=cb3_sb, in_=cb3_in)
        xpad_v = xpad[:, 0:NPIX].rearrange("p (r s) -> p r s", r=GP)
        nc.vector.tensor_copy(
            out=xpad_v[:, PADG:PADG + H, PADG:PADG + W],
            in_=xbf.rearrange("p (h w) -> p h w", h=H),
        )
        xq4 = singles.tile([128, NPIX, 4], BF16)
        for j, d in enumerate((0, 1, GP, GP + 1)):
            eng = nc.scalar if j % 2 == 0 else nc.vector
            if eng is nc.scalar:
                nc.scalar.activation(out=xq4[:, :, j], in_=xpad[:, d:d + NPIX], func=AF.Copy)
            else:
                nc.vector.tensor_copy(out=xq4[:, :, j], in_=xpad[:, d:d + NPIX])

        # ---- offset conv: offs [18, 4096] f32 ----
        offs = pa.tile([18, HWP], F32)
        with tc.tile_pool(name="poff", bufs=3, space="PSUM") as poffp:
            for ic in range(8):
                poff = poffp.tile([18, 512], F32)
                h0 = ic * 8
                for t in range(9):
                    ty, tx = t // 3, t % 3
                    rhs = xpad_v[:, 2 + ty + h0:2 + ty + h0 + 8, 2 + tx:2 + tx + W]
                    nc.tensor.matmul(poff, lhsT=wof_sb[:, t, :], rhs=rhs,
                                     start=(t == 0), stop=(t == 8))
                nc.scalar.activation(out=offs[:, ic * 512:(ic + 1) * 512], in_=poff,
                                     func=AF.Identity, bias=boff_sb, scale=1.0)

        # ---- index / weight prep in [72, 1024] packed layout ----
        # offsP rows p=k*4+a: y rows 0..35, x rows 36..71; i = a*1024 + ii
        prep_cm = tc.tile_pool(name="prep", bufs=1)
        prep = prep_cm.__enter__()
        offsP = prep.tile([36, 2048], F32)
        nc.sync.dma_start(
            out=offsP[:, 0:1024],
            in_=offs[0:9, :].rearrange("p (a f) -> p a f", a=4))
        nc.scalar.dma_start(
            out=offsP[:, 1024:2048],
            in_=offs[9:18, :].rearrange("p (a f) -> p a f", a=4))

        nc.vector.tensor_tensor(out=offsP, in0=offsP, in1=cb3_sb, op=ALU.add)
        nc.vector.tensor_scalar(out=offsP, in0=offsP, scalar1=CLAMP_LO, scalar2=CLAMP_HI,
                                op0=ALU.max, op1=ALU.min)
        p3c = offsP
        flr_i = prep.tile([36, 2048], I16)
        nc.scalar.activation(out=flr_i, in_=p3c, func=AF.Copy, bias=-FLOOR_EPS)
        flr = prep.tile([36, 2048], F32)
        nc.scalar.activation(out=flr, in_=flr_i, func=AF.Copy)
        frac = prep.tile([36, 2048], F32)
        nc.vector.tensor_tensor(out=frac, in0=p3c, in1=flr, op=ALU.subtract)
        omf = prep.tile([36, 2048], F32)
        nc.vector.tensor_scalar(out=omf, in0=frac, scalar1=-1.0, scalar2=1.0,
                                op0=ALU.mult, op1=ALU.add)

        # bilinear weights, j-interleaved to match xq4/gather layout
        w4s = singles.tile([36, 1024, 4], BF16)
        nc.vector.tensor_tensor(out=w4s[:, :, 0], in0=omf[:, 0:1024], in1=omf[:, 1024:2048], op=ALU.mult)
        nc.vector.tensor_tensor(out=w4s[:, :, 1], in0=omf[:, 0:1024], in1=frac[:, 1024:2048], op=ALU.mult)
        nc.vector.tensor_tensor(out=w4s[:, :, 2], in0=frac[:, 0:1024], in1=omf[:, 1024:2048], op=ALU.mult)
        nc.vector.tensor_tensor(out=w4s[:, :, 3], in0=frac[:, 0:1024], in1=frac[:, 1024:2048], op=ALU.mult)

        # flat gather index = y0p*70 + x0p (into the padded 70x70 grid)
        idxf = prep.tile([36, 1024], F32)
        nc.vector.tensor_scalar(out=idxf, in0=flr[:, 0:1024], scalar1=float(GP), scalar2=None,
                                op0=ALU.mult)
        nc.vector.tensor_tensor(out=idxf, in0=idxf, in1=flr[:, 1024:2048], op=ALU.add)
        # cast to int16 replicated 8x in SBUF (reusing the dead offs slot), so the
        # DRAM write below is fully contiguous (16KB runs, no 32B-descriptor storm)
        idx16r = pa.tile([36, 64, 8, 16], I16, tag="offs", name="idx16r")
        idxf_v = idxf.rearrange("p (s r) -> p s r", r=16)
        for k8 in range(8):
            if k8 % 2 == 0:
                nc.scalar.activation(out=idx16r[:, :, k8, :], in_=idxf_v, func=AF.Copy)
            else:
                nc.vector.tensor_copy(out=idx16r[:, :, k8, :], in_=idxf_v)

        # roundtrip through DRAM with each 16-idx chunk replicated 8x, so ONE
        # wide xbar transpose [2304,128]->[128,2304] yields the wrapped layout
        # for all 8 gather groups (16-col transposes fall on the degenerate
        # AP-swap path: in free dim 16 < XBAR_TILE_SRC_COLS=128).
        idxd = dram.tile([2304, 128], I16)
        nc.sync.dma_start(out=idxd, in_=idx16r)
        prep_cm.__exit__(None, None, None)
        pa_cm.__exit__(None, None, None)
        idxw = singles.tile([128, 2304], I16)
        nc.sync.dma_start_transpose(idxw, idxd[:, :])

        # ---- main loop: gather -> weight -> GEMM, per (quarter, tap) ----
        out_sb = [singles.tile([128, HWP], F32, name=f"out_sb{i}") for i in range(2)]
        short_sb = [singles.tile([128, HWP], BF16, name=f"short_sb{i}") for i in range(2)]
        stats_sb = singles.tile([128, 2, 8, 6], F32)

        gpool = ctx.enter_context(tc.tile_pool(name="gpool", bufs=3))
        wstp = ctx.enter_context(tc.tile_pool(name="wstp", bufs=3))
        tpool = ctx.enter_context(tc.tile_pool(name="tpool", bufs=3))
        accp = ctx.enter_context(tc.tile_pool(name="accp", bufs=1, space="PSUM"))
        pwp = ctx.enter_context(tc.tile_pool(name="pwp", bufs=2, space="PSUM"))
        shp = ctx.enter_context(tc.tile_pool(name="shp", bufs=1, space="PSUM"))

        for iq in range(4):
            acc = [accp.tile([128, 1024], F32, tag="acc0", name="acc0"),
                   accp.tile([128, 1024], F32, tag="acc1", name="acc1")]
            for k in range(9):
                g4 = gpool.tile([128, 1024, 4], BF16)
                s0 = k * 256 + iq * 64
                if not _BUILD_OPTS["skip_gather"]:
                    nc.gpsimd.ap_gather(g4, xq4, idxw[:, s0:s0 + 64],
                                        channels=128, num_elems=NPIX, d=4, num_idxs=1024)
                else:
                    nc.vector.memset(g4, 0.5)
                t4 = tpool.tile([128, 1024, 4], BF16)
                g4f = g4.rearrange("p a b -> p (a b)")
                t4f = t4.rearrange("p a b -> p (a b)")
                w4row = w4s[4 * k + iq:4 * k + iq + 1, :, :].rearrange("p a b -> p (a b)")
                if not _BUILD_OPTS["skip_wst"]:
                    wst = wstp.tile([1, 4096], BF16, name="wst")
                    nc.sync.dma_start(out=wst, in_=w4row)
                if _BUILD_OPTS["skip_repl_mul"]:
                    nc.vector.tensor_copy(out=t4f, in_=g4f)
                else:
                    for h in range(8):
                        pw4 = pwp.tile([128, 512], F32)
                        nc.tensor.matmul(pw4, lhsT=ones_sb,
                                         rhs=wst[:, 512 * h:512 * (h + 1)],
                                         start=True, stop=True)
                        nc.vector.tensor_tensor(out=t4f[:, 512 * h:512 * (h + 1)],
                                                in0=g4f[:, 512 * h:512 * (h + 1)],
                                                in1=pw4, op=ALU.mult)
                for ob in range(2):
                    jr = range(1) if _BUILD_OPTS["skip_gemm"] else range(4)
                    for j in jr:
                        for c2 in range(2):
                            nc.tensor.matmul(
                                acc[ob][:, 512 * c2:512 * (c2 + 1)],
                                lhsT=wdef_sb[:, k, 128 * ob:128 * (ob + 1)],
                                rhs=t4[:, 512 * c2:512 * (c2 + 1), j],
                                start=(k == 0 and j == 0),
                                stop=(k == 8 and j == (0 if _BUILD_OPTS["skip_gemm"] else 3)))
            for ob in range(2):
                for c2 in range(2):
                    nc.vector.bn_stats(out=stats_sb[:, ob, 2 * iq + c2, :],
                                       in_=acc[ob][:, 512 * c2:512 * (c2 + 1)])
                shortp = shp.tile([128, 1024], F32, tag="short")
                for c2 in range(2):
                    nc.tensor.matmul(shortp[:, 512 * c2:512 * (c2 + 1)],
                                     lhsT=wsc_sb[:, 128 * ob:128 * (ob + 1)],
                                     rhs=xbf[:, 1024 * iq + 512 * c2:1024 * iq + 512 * (c2 + 1)],
                                     start=True, stop=True)
                nc.scalar.activation(out=out_sb[ob][:, 1024 * iq:1024 * (iq + 1)],
                                     in_=acc[ob], func=AF.Copy)
                nc.scalar.activation(out=short_sb[ob][:, 1024 * iq:1024 * (iq + 1)],
                                     in_=shortp, func=AF.Copy)

        # ---- BN stats: per-core sums -> AllReduce -> scale/shift ----
        sums = singles.tile([128, 4], F32)  # [S1_ob0, S2_ob0, S1_ob1, S2_ob1]
        mvt = singles.tile([128, 2, 2], F32)
        for ob in range(2):
            nc.vector.bn_aggr(out=mvt[:, ob, :], in_=stats_sb[:, ob, :, :])
            # S1 = mean*4096 ; S2 = (var + mean^2)*4096
            nc.vector.tensor_scalar(out=sums[:, 2 * ob:2 * ob + 1], in0=mvt[:, ob, 0:1],
                                    scalar1=float(HWP), scalar2=None, op0=ALU.mult)
            msq = singles.tile([128, 1], F32, tag=f"msq{ob}")
            nc.vector.tensor_tensor(out=msq, in0=mvt[:, ob, 0:1], in1=mvt[:, ob, 0:1],
                                    op=ALU.mult)
            nc.vector.tensor_tensor(out=msq, in0=msq, in1=mvt[:, ob, 1:2], op=ALU.add)
            nc.vector.tensor_scalar(out=sums[:, 2 * ob + 1:2 * ob + 2], in0=msq,
                                    scalar1=float(HWP), scalar2=None, op0=ALU.mult)

        ccin = dram.tile([128, 4], F32)
        ccout = dram.tile([128, 4], F32)
        nc.sync.dma_start(out=ccin, in_=sums)
        if not _BUILD_OPTS["skip_collective"]:
            nc.gpsimd.collective_compute(
                "AllReduce", ALU.add, replica_groups=[list(range(8))],
                ins=[ccin.opt()], outs=[ccout.opt()])
        else:
            nc.sync.dma_start(out=ccout, in_=ccin)
        gsums = singles.tile([128, 4], F32)
        nc.sync.dma_start(out=gsums, in_=ccout)

        NTOT = float(B * HWP)
        scale = singles.tile([128, 2], F32)
        shift = singles.tile([128, 2], F32)
        for ob in range(2):
            mean_g = singles.tile([128, 1], F32, tag=f"mg{ob}")
            nc.vector.tensor_scalar(out=mean_g, in0=gsums[:, 2 * ob:2 * ob + 1],
                                    scalar1=1.0 / NTOT, scalar2=None, op0=ALU.mult)
            var_g = singles.tile([128, 1], F32, tag=f"vg{ob}")
            nc.vector.tensor_scalar(out=var_g, in0=gsums[:, 2 * ob + 1:2 * ob + 2],
                                    scalar1=1.0 / NTOT, scalar2=None, op0=ALU.mult)
            msq2 = singles.tile([128, 1], F32, tag=f"msq2{ob}")
            nc.vector.tensor_tensor(out=msq2, in0=mean_g, in1=mean_g, op=ALU.mult)
            nc.vector.tensor_tensor(out=var_g, in0=var_g, in1=msq2, op=ALU.subtract)
            # rstd = 1/sqrt(var+eps)
            sd = singles.tile([128, 1], F32, tag=f"sd{ob}")
            nc.scalar.activation(out=sd, in_=var_g, func=AF.Sqrt, bias=eps_sb, scale=1.0)
            rstd = singles.tile([128, 1], F32, tag=f"rs{ob}")
            nc.vector.reciprocal(out=rstd, in_=sd)
            nc.vector.tensor_tensor(out=scale[:, ob:ob + 1], in0=gam_sb[:, ob:ob + 1],
                                    in1=rstd, op=ALU.mult)
            sm = singles.tile([128, 1], F32, tag=f"sm{ob}")
            nc.vector.tensor_tensor(out=sm, in0=scale[:, ob:ob + 1], in1=mean_g, op=ALU.mult)
            nc.vector.tensor_tensor(out=shift[:, ob:ob + 1], in0=bsc_sb[:, ob:ob + 1],
                                    in1=sm, op=ALU.subtract)

        # ---- final: relu(scale*acc + shift + short) ----
        finp = ctx.enter_context(tc.tile_pool(name="finp", bufs=2))
        FC = 1024
        for ob in range(2):
            for c in range(HWP // FC):
                sl = slice(FC * c, FC * (c + 1))
                fin = finp.tile([128, FC], F32, tag="fin", name="fin")
                nc.vector.tensor_scalar(out=fin, in0=out_sb[ob][:, sl],
                                        scalar1=scale[:, ob:ob + 1],
                                        scalar2=shift[:, ob:ob + 1], op0=ALU.mult, op1=ALU.add)
                nc.vector.tensor_tensor(out=fin, in0=fin, in1=short_sb[ob][:, sl], op=ALU.add)
                fin2 = finp.tile([128, FC], F32, tag="fin2", name="fin2")
                nc.scalar.activation(out=fin2, in_=fin, func=AF.Relu)
                nc.sync.dma_start(out=out_d[ob, :, sl], in_=fin2)

    nc.compile()
    return nc


_NC_CACHE = {}


def _get_program():
    if "nc" not in _NC_CACHE:
        _NC_CACHE["nc"] = _build_program()
    return _NC_CACHE["nc"]


def _host_prep(w_off, b_off, w_def, b_def, gamma, beta, w_sc, b_sc):
    bf = ml_dtypes.bfloat16
    w_off = np.asarray(w_off, np.float32)
    w_def = np.asarray(w_def, np.float32)
    w_sc = np.asarray(w_sc, np.float32)

    # wof [128, 9, 18]: col j = dy channel (2j), col 9+j = dx channel (2j+1), tap t=(ty,tx)
    wof = np.empty((128, 9, 18), np.float32)
    wr = w_off.reshape(9, 2, CIN, 3, 3)
    for t in range(9):
        ty, tx = t // 3, t % 3
        wof[:, t, 0:9] = wr[:, 0, :, ty, tx].T
        wof[:, t, 9:18] = wr[:, 1, :, ty, tx].T
    boff = np.concatenate([b_off[0::2], b_off[1::2]]).reshape(18, 1).astype(np.float32)

    wdef = np.ascontiguousarray(w_def.transpose(2, 3, 1, 0).reshape(9, CIN, COUT)
                                .transpose(1, 0, 2))  # [128, 9, 256]
    wsc = np.ascontiguousarray(w_sc[:, :, 0, 0].T)  # [128, 256]

    cb3 = np.ones((36, 2048), np.float32)
    ii = np.arange(1024)
    for k in range(9):
        ky, kx = k // 3 - 1, k % 3 - 1
        for a in range(4):
            i = a * 1024 + ii
            cb3[4 * k + a, 0:1024] = (i // W) + ky + PADG
            cb3[4 * k + a, 1024:2048] = (i % W) + kx + PADG

    gamma2 = np.ascontiguousarray(np.asarray(gamma, np.float32).reshape(2, 128).T)
    betasc2 = np.ascontiguousarray(
        (np.asarray(beta, np.float32) + np.asarray(b_sc, np.float32)).reshape(2, 128).T)

    return {
        "wof": wof.astype(bf), "boff": boff, "wdef": wdef.astype(bf),
        "wsc": wsc.astype(bf), "cb3": cb3, "onesb": np.ones((1, 128), bf),
        "gamma2": gamma2, "betasc2": betasc2,
    }


def run(inputs, trace=False):
    nc = _get_program()
    x = np.asarray(inputs["x"], np.float32)
    consts = _host_prep(
        inputs["w_off"], inputs["b_off"], inputs["w_def"], inputs["b_def"],
        inputs["gamma"], inputs["beta"], inputs["w_sc"], inputs["b_sc"])
    in_maps = []
    for b in range(B):
        m = dict(consts)
        m["x"] = np.ascontiguousarray(x[b].reshape(CIN, HWP))
        in_maps.append(m)
    try:
        r = run_bass_kernel_spmd(nc, in_maps, list(range(8)), trace=trace)
    except ModuleNotFoundError:
        # NTFF trace hook unavailable in this container; run untraced
        r = run_bass_kernel_spmd(nc, in_maps, list(range(8)), trace=False)
    out = np.stack([r.results[b]["out"].reshape(COUT, H, W) for b in range(B)])
    return out.astype(np.float32), r


def kernel(**inputs):
    out, _ = run(inputs)
    return out


def bench(inputs, reps=30):
    """Time repeated on-device executions (async dispatch, single sync)."""
    import jax
    from jax.sharding import Mesh, PartitionSpec
    from jax.experimental.shard_map import shard_map
    import concourse.mybir as _mybir
    from concourse import bass2jax
    import time as _time

    nc = _get_program()
    bass2jax.install_neuronx_cc_hook()
    x = np.asarray(inputs["x"], np.float32)
    consts = _host_prep(
        inputs["w_off"], inputs["b_off"], inputs["w_def"], inputs["b_def"],
        inputs["gamma"], inputs["beta"], inputs["w_sc"], inputs["b_sc"])
    in_maps = []
    for b in range(B):
        m = dict(consts)
        m["x"] = np.ascontiguousarray(x[b].reshape(CIN, HWP))
        in_maps.append(m)

    in_names, out_names, out_avals, zero_outs = [], [], [], []
    for alloc in nc.m.functions[0].allocations:
        if not isinstance(alloc, _mybir.MemoryLocationSet):
            continue
        name = alloc.memorylocations[0].name
        if alloc.kind == "ExternalInput":
            if nc.partition_id_tensor is None or name != nc.partition_id_tensor.name:
                in_names.append(name)
        elif alloc.kind == "ExternalOutput":
            out_names.append(name)
            shape = tuple(alloc.tensor_shape)
            dtype = _mybir.dt.np(alloc.dtype)
            out_avals.append(jax.core.ShapedArray(shape, dtype))
            zero_outs.append(np.zeros(shape, dtype))
    n_params = len(in_names)
    in_names_full = in_names + out_names
    if nc.partition_id_tensor is not None:
        in_names_full = in_names_full + [nc.partition_id_tensor.name]

    def _body(*args):
        operands = list(args)
        if nc.partition_id_tensor is not None:
            operands.append(bass2jax.partition_id_tensor())
        outs = bass2jax._bass_exec_p.bind(
            *operands,
            out_avals=tuple(out_avals),
            in_names=tuple(in_names_full),
            out_names=tuple(out_names),
            lowering_input_output_aliases=(),
            sim_require_finite=True,
            sim_require_nnan=True,
            nc=nc,
        )
        return tuple(outs)

    devices = jax.devices()[:8]
    mesh = Mesh(np.asarray(devices), ("core",))
    n_outs = len(out_names)
    sharded = jax.jit(
        shard_map(_body, mesh=mesh,
                  in_specs=(PartitionSpec("core"),) * (n_params + n_outs),
                  out_specs=(PartitionSpec("core"),) * n_outs,
                  check_rep=False),
        keep_unused=True,
    )
    per_core = [[np.asarray(m[nm]) for nm in in_names] for m in in_maps]
    concat_in = [np.concatenate([per_core[c][i] for c in range(8)], axis=0)
                 for i in range(n_params)]
    concat_zeros = [np.zeros((8 * z.shape[0], *z.shape[1:]), z.dtype) for z in zero_outs]
    from jax.sharding import NamedSharding
    sh = NamedSharding(mesh, PartitionSpec("core"))
    args = [jax.device_put(a, sh) for a in concat_in + concat_zeros]

    o = sharded(*args)  # compile + warmup
    jax.block_until_ready(o)
    o = sharded(*args)
    jax.block_until_ready(o)

    t0 = _time.time()
    outs = [sharded(*args) for _ in range(reps)]
    jax.block_until_ready(outs)
    dt = (_time.time() - t0) / reps
    return dt

